# revision 4
# baseline (speedup 1.0000x reference)
"""Trainium2 Bass kernel for the gated-MLP-over-ring-buffer problem.

Reference computation (B=512, M=128, V=256, H=256, IN = M*V = 32768):
    mem    = roll(memory, 1, axis=1); mem[:, 0, :] = x        # [B, M, V]
    flat   = mem.reshape(B, IN)                                # [B, 32768]
    h      = tanh(flat @ W1 + b1) * sigmoid(flat @ Wg + bg)    # [B, 256]
    logits = h @ W2 + b2                                       # [B, 256]

Strategy (8 NeuronCores, one trn2 chip):
  - Contraction-shard the two big GEMMs: core c owns k-rows
    [4096c, 4096(c+1)) of W1/Wg and the matching slab of flat.T
    (host-prepared, transposed + bf16 so SBUF tiles load at line rate,
    partition-major so each k-group is one contiguous DMA).
  - Each core computes partial P1.T / Pg.T = W.T @ flat.T  -> [H, B]
    accumulated over its 32 k-chunks in PSUM (bf16 operands, f32 acc).
  - Cross-core reduction of the [2H, B] partials in bf16, scattered
    over B (AllToAll) so core c ends up with batch cols [64c, 64c+64).
    Split in two K-halves so the first AllToAll hides under the second
    half of compute; a dummy AllToAll at kernel start absorbs the
    one-time collective barrier / control-plane warmup.
  - Each core applies bias + tanh/sigmoid gating and the small W2
    GEMM (bf16) for its batch chunk, writing logits.T [V, 64].
  - Host assembles/transposes the 8 chunks back to [B, V].
"""

import numpy as np

import concourse.bacc as bacc
import concourse.bass as bass
import concourse.mybir as mybir
import concourse.tile as tile
from concourse import bass_utils

B, M, V, H = 512, 128, 256, 256
IN = M * V              # 32768
NCORES = 8
KC = IN // NCORES       # 4096 contraction rows per core
NKG = 8                 # DMA k-groups per core
KB_PER_G = KC // (NKG * 128)  # 4 k-chunks of 128 per group
BCHUNK = B // NCORES    # 64 batch columns per core after reduce-scatter
SPLIT_G = 4             # k-groups in the first (early-flushed) half
WARMUP_MM = 12

F32 = mybir.dt.float32
BF16 = mybir.dt.bfloat16
AF = mybir.ActivationFunctionType
RG = [list(range(NCORES))]

_CACHE = {}


def _build():
    nc = bacc.Bacc(
        "TRN2",
        target_bir_lowering=False,
        debug=False,
        enable_asserts=False,
        num_devices=NCORES,
    )

    # Per-core external inputs (host-packed, partition-major, bf16).
    memT = nc.dram_tensor("memT", [NKG, 128, KB_PER_G, B], BF16, kind="ExternalInput")
    wpk = nc.dram_tensor("wpk", [NKG, 128, KB_PER_G, 2, H], BF16, kind="ExternalInput")
    w2pk = nc.dram_tensor("w2pk", [128, 2, V], BF16, kind="ExternalInput")
    # packed biases: cols = [b1_lo, b1_hi, bg_lo, bg_hi, b2_lo, b2_hi]
    bpk = nc.dram_tensor("bpk", [128, 6], F32, kind="ExternalInput")
    outT = nc.dram_tensor("outT", [V, BCHUNK], F32, kind="ExternalOutput")

    with tile.TileContext(nc) as tc:
        with (
            tc.tile_pool(name="xg", bufs=NKG) as xpool,
            tc.tile_pool(name="wt", bufs=NKG) as wpool,
            tc.tile_pool(name="part", bufs=1) as ppool,
            tc.tile_pool(name="s2", bufs=1) as s2pool,
            tc.tile_pool(name="psum1", bufs=1, space="PSUM") as psum1,
            tc.tile_pool(name="dram", bufs=1, space="DRAM") as dpool,
        ):
            # ---- dummy collective: warms the CC stream / rank barrier ----
            dumin = dpool.tile([NCORES, 256], BF16, tag="dumin", name="dumin")
            dumout = dpool.tile([NCORES, 256], BF16, tag="dumout", name="dumout")
            dum = s2pool.tile([NCORES, 256], BF16, tag="dum")
            nc.gpsimd.memset(dum[:], 0.0)
            nc.sync.dma_start(out=dumin[:], in_=dum[:])
            nc.gpsimd.collective_compute(
                "AllToAll",
                mybir.AluOpType.bypass,
                replica_groups=RG,
                ins=[dumin[:].opt()],
                outs=[dumout[:].opt()],
            )

            # Stage-2 constants on the (otherwise idle) gpsimd SWDGE queue.
            bt = s2pool.tile([128, 6], F32, tag="bias")
            nc.gpsimd.dma_start(out=bt[:], in_=bpk[:, :])
            w2t = s2pool.tile([128, 2, V], BF16, tag="w2")
            nc.gpsimd.dma_start(out=w2t[:], in_=w2pk[:, :, :])

            # Pre-warm the PE HAM clock gate with dummy matmuls while the
            # first input DMAs are in flight.
            wsrc = s2pool.tile([128, B], BF16, tag="wsrc")
            nc.gpsimd.memset(wsrc[:], 0.0)
            wps = psum1.tile([128, B], F32, tag="acc7", name="wps")
            for i in range(WARMUP_MM):
                nc.tensor.matmul(
                    wps[:],
                    wsrc[:, 0:128],
                    wsrc[:],
                    start=(i == 0),
                    stop=(i == WARMUP_MM - 1),
                )

            # ---------------- stage 1: partial W.T @ flat.T ----------------
            acc = [
                [
                    psum1.tile([128, B], F32, tag=f"acc{4 * s + i}", name=f"acc{s}_{i}")
                    for i in range(4)
                ]
                for s in range(2)
            ]
            ccin = [None, None]
            ccout = [None, None]
            for s in range(2):
                ccin[s] = dpool.tile(
                    [NCORES, 128, 4, BCHUNK], BF16, tag=f"ccin{s}", name=f"ccin{s}"
                )
                ccout[s] = dpool.tile(
                    [NCORES, 128, 4, BCHUNK], BF16, tag=f"ccout{s}", name=f"ccout{s}"
                )

            def flush_half(s):
                # PSUM -> SBUF (cast bf16, c-major) -> DRAM -> AllToAll
                sb = ppool.tile(
                    [128, NCORES, 4, BCHUNK], BF16, tag=f"sb{s}", name=f"sb{s}"
                )
                for t in range(4):
                    nc.vector.tensor_copy(
                        sb[:, :, t, :],
                        acc[s][t][:].rearrange("p (c b) -> p c b", c=NCORES),
                    )
                nc.sync.dma_start(
                    out=ccin[s][:].rearrange("c p t b -> p c (t b)"),
                    in_=sb[:].rearrange("p c t b -> p c (t b)"),
                )
                nc.gpsimd.collective_compute(
                    "AllToAll",
                    mybir.AluOpType.bypass,
                    replica_groups=RG,
                    ins=[ccin[s][:].opt()],
                    outs=[ccout[s][:].opt()],
                )

            for kg in range(NKG):
                s = 0 if kg < SPLIT_G else 1
                xg = xpool.tile([128, KB_PER_G, B], BF16, tag="xg")
                nc.sync.dma_start(out=xg[:], in_=memT[kg])
                wt = wpool.tile([128, KB_PER_G, 2, H], BF16, tag="wt")
                nc.scalar.dma_start(out=wt[:], in_=wpk[kg])
                for kb in range(KB_PER_G):
                    k = kg * KB_PER_G + kb
                    lo = 0 if s == 0 else SPLIT_G * KB_PER_G
                    hi = SPLIT_G * KB_PER_G if s == 0 else NKG * KB_PER_G
                    first = k == lo
                    last = k == hi - 1
                    rhs = xg[:, kb, :]
                    for w in range(2):
                        for h in range(2):
                            nc.tensor.matmul(
                                acc[s][2 * w + h][:],
                                wt[:, kb, w, bass.ts(h, 128)],
                                rhs,
                                start=first,
                                stop=last,
                            )
                if kg == 1:
                    # Pre-warm the Tanh/Sigmoid activation tables while the
                    # scalar engine is otherwise idle (first use of each
                    # table pays a ~1.3us load).
                    warm = s2pool.tile([128, 1], F32, tag="warm")
                    nc.gpsimd.memset(warm[:], 0.0)
                    warm2 = s2pool.tile([128, 1], F32, tag="warm2")
                    nc.scalar.activation(warm2[:], warm[:], AF.Tanh)
                    nc.scalar.activation(warm[:], warm2[:], AF.Sigmoid)
                if kg == SPLIT_G - 1:
                    flush_half(0)
            flush_half(1)

            # ------------- local reduction of received slabs -------------
            t2 = [None, None]
            for s in range(2):
                rr = s2pool.tile(
                    [128, NCORES, 4, BCHUNK], BF16, tag=f"rr{s}", name=f"rr{s}"
                )
                nc.sync.dma_start(
                    out=rr[:].rearrange("p c t b -> p c (t b)"),
                    in_=ccout[s][:].rearrange("c p t b -> p c (t b)"),
                )
                t1 = s2pool.tile([128, 4, 4, BCHUNK], F32, tag=f"t1{s}", name=f"t1{s}")
                nc.vector.tensor_add(t1[:], rr[:, 0:4, :, :], rr[:, 4:8, :, :])
                t2[s] = s2pool.tile(
                    [128, 2, 4, BCHUNK], F32, tag=f"t2{s}", name=f"t2{s}"
                )
                nc.vector.tensor_add(t2[s][:], t1[:, 0:2, :, :], t1[:, 2:4, :, :])
            s2a = s2pool.tile([128, 2, 4, BCHUNK], F32, tag="s2a")
            nc.vector.tensor_add(s2a[:], t2[0][:], t2[1][:])
            s2 = s2pool.tile([128, 4, BCHUNK], F32, tag="s2in")
            nc.vector.tensor_add(s2[:], s2a[:, 0, :, :], s2a[:, 1, :, :])

            # ---------------- stage 2: gate + W2 ----------------
            hT = []
            for i in range(2):
                th = s2pool.tile([128, BCHUNK], BF16, tag=f"th{i}", name=f"th{i}")
                nc.scalar.activation(th[:], s2[:, i, :], AF.Tanh, bias=bt[:, i : i + 1])
                sg = s2pool.tile([128, BCHUNK], BF16, tag=f"sg{i}", name=f"sg{i}")
                nc.scalar.activation(
                    sg[:], s2[:, 2 + i, :], AF.Sigmoid, bias=bt[:, 2 + i : 3 + i]
                )
                ht = s2pool.tile([128, BCHUNK], BF16, tag=f"ht{i}", name=f"ht{i}")
                nc.vector.tensor_mul(ht[:], th[:], sg[:])
                hT.append(ht)

            for v in range(2):
                ps = psum1.tile([128, BCHUNK], F32, tag=f"acc{v}", name=f"ps2_{v}")
                for i in range(2):
                    nc.tensor.matmul(
                        ps[:],
                        w2t[:, i, bass.ts(v, 128)],
                        hT[i][:],
                        start=(i == 0),
                        stop=(i == 1),
                    )
                ot = s2pool.tile([128, BCHUNK], F32, tag=f"ot{v}", name=f"ot{v}")
                nc.vector.tensor_scalar_add(ot[:], ps[:], bt[:, 4 + v : 5 + v])
                nc.sync.dma_start(out=outT[bass.ts(v, 128), :], in_=ot[:])

    nc.compile()
    return nc


def _shard(x, memory, W1, b1, Wg, bg, W2, b2):
    """Build the 8 per-core input maps from the full problem inputs."""
    import ml_dtypes

    bf16 = ml_dtypes.bfloat16
    x = np.asarray(x, dtype=np.float32)
    memory = np.asarray(memory, dtype=np.float32)
    W1 = np.asarray(W1, dtype=np.float32)
    Wg = np.asarray(Wg, dtype=np.float32)
    W2 = np.asarray(W2, dtype=np.float32)
    b1 = np.asarray(b1, dtype=np.float32)
    bg = np.asarray(bg, dtype=np.float32)
    b2 = np.asarray(b2, dtype=np.float32)

    # rolled ring buffer, flattened and transposed: [IN, B]
    flatT = np.empty((IN, B), dtype=bf16)
    flatT[:V] = x.T
    flatT[V:] = memory[:, : M - 1, :].reshape(B, IN - V).T
    bpk = np.ascontiguousarray(
        np.stack([b1[:128], b1[128:], bg[:128], bg[128:], b2[:128], b2[128:]], axis=1)
    )
    w2pk = np.ascontiguousarray(
        W2.reshape(2, 128, V).transpose(1, 0, 2).astype(bf16)
    )

    in_maps = []
    for c in range(NCORES):
        sl = slice(KC * c, KC * (c + 1))
        # [NKG, KB, 128, B] -> partition-major [NKG, 128, KB, B]
        mT = np.ascontiguousarray(
            flatT[sl].reshape(NKG, KB_PER_G, 128, B).transpose(0, 2, 1, 3)
        )
        w1s = W1[sl].reshape(NKG, KB_PER_G, 128, H)
        wgs = Wg[sl].reshape(NKG, KB_PER_G, 128, H)
        wpk = np.ascontiguousarray(
            np.stack([w1s, wgs], axis=3).transpose(0, 2, 1, 3, 4).astype(bf16)
        )
        in_maps.append(
            {"memT": mT, "wpk": wpk, "w2pk": w2pk, "bpk": bpk}
        )
    return in_maps


def _get_nc():
    if "nc" not in _CACHE:
        _CACHE["nc"] = _build()
    return _CACHE["nc"]


def kernel(x, memory, W1, b1, Wg, bg, W2, b2, **run_kwargs):
    nc = _get_nc()
    in_maps = _shard(x, memory, W1, b1, Wg, bg, W2, b2)
    res = bass_utils.run_bass_kernel_spmd(
        nc, in_maps, core_ids=list(range(NCORES)), **run_kwargs
    )
    _CACHE["last_results"] = res
    out = np.empty((B, V), dtype=np.float32)
    for c in range(NCORES):
        out[c * BCHUNK : (c + 1) * BCHUNK, :] = res.results[c]["outT"].T
    return out


# revision 6
# speedup vs baseline: 1.0748x; 1.0748x over previous
"""Trainium2 Bass kernel for the gated-MLP-over-ring-buffer problem.

Reference computation (B=512, M=128, V=256, H=256, IN = M*V = 32768):
    mem    = roll(memory, 1, axis=1); mem[:, 0, :] = x        # [B, M, V]
    flat   = mem.reshape(B, IN)                                # [B, 32768]
    h      = tanh(flat @ W1 + b1) * sigmoid(flat @ Wg + bg)    # [B, 256]
    logits = h @ W2 + b2                                       # [B, 256]

Strategy (8 NeuronCores, one trn2 chip):
  - Contraction-shard the two big GEMMs: core c owns k-rows
    [4096c, 4096(c+1)) of W1/Wg and the matching slab of flat.T
    (host-prepared, transposed + bf16 so SBUF tiles load at line rate,
    partition-major so each k-group is one contiguous DMA).
  - Each core computes partial P1.T / Pg.T = W.T @ flat.T  -> [H, B]
    accumulated over its 32 k-chunks in PSUM (bf16 operands, f32 acc).
  - Cross-core reduction of the [2H, B] partials in bf16, scattered
    over B (AllToAll) so core c ends up with batch cols [64c, 64c+64).
    Split in two K-halves so the first AllToAll hides under the second
    half of compute; a dummy AllToAll at kernel start absorbs the
    one-time collective barrier / control-plane warmup.
  - Each core applies bias + tanh/sigmoid gating and the small W2
    GEMM (bf16) for its batch chunk, writing logits.T [V, 64].
  - Host assembles/transposes the 8 chunks back to [B, V].
"""

import numpy as np

import concourse.bacc as bacc
import concourse.bass as bass
import concourse.mybir as mybir
import concourse.tile as tile
from concourse import bass_utils

B, M, V, H = 512, 128, 256, 256
IN = M * V              # 32768
NCORES = 8
KC = IN // NCORES       # 4096 contraction rows per core
NKG = 8                 # DMA k-groups per core
KB_PER_G = KC // (NKG * 128)  # 4 k-chunks of 128 per group
BCHUNK = B // NCORES    # 64 batch columns per core after reduce-scatter
SPLIT_G = 4             # k-groups in the first (early-flushed) half
WARMUP_MM = 12

F32 = mybir.dt.float32
BF16 = mybir.dt.bfloat16
AF = mybir.ActivationFunctionType
RG = [list(range(NCORES))]

_CACHE = {}


def _build():
    nc = bacc.Bacc(
        "TRN2",
        target_bir_lowering=False,
        debug=False,
        enable_asserts=False,
        num_devices=NCORES,
    )

    # Per-core external inputs (host-packed, partition-major, bf16).
    memT = nc.dram_tensor("memT", [NKG, 128, KB_PER_G, B], BF16, kind="ExternalInput")
    wpk = nc.dram_tensor("wpk", [NKG, 128, KB_PER_G, 2, H], BF16, kind="ExternalInput")
    w2pk = nc.dram_tensor("w2pk", [128, 2, V], BF16, kind="ExternalInput")
    # packed biases: cols = [b1_lo, b1_hi, bg_lo, bg_hi, b2_lo, b2_hi]
    bpk = nc.dram_tensor("bpk", [128, 6], F32, kind="ExternalInput")
    outT = nc.dram_tensor("outT", [V, BCHUNK], F32, kind="ExternalOutput")

    with tile.TileContext(nc) as tc:
        with (
            tc.tile_pool(name="xg", bufs=NKG) as xpool,
            tc.tile_pool(name="wt", bufs=NKG) as wpool,
            tc.tile_pool(name="part", bufs=1) as ppool,
            tc.tile_pool(name="s2", bufs=1) as s2pool,
            tc.tile_pool(name="psum1", bufs=1, space="PSUM") as psum1,
            tc.tile_pool(name="dram", bufs=1, space="DRAM") as dpool,
        ):
            # Stage-2 constants on the (otherwise idle) gpsimd SWDGE queue.
            bt = s2pool.tile([128, 6], F32, tag="bias")
            nc.gpsimd.dma_start(out=bt[:], in_=bpk[:, :])
            w2t = s2pool.tile([128, 2, V], BF16, tag="w2")
            nc.gpsimd.dma_start(out=w2t[:], in_=w2pk[:, :, :])

            # Pre-warm the PE HAM clock gate with dummy matmuls while the
            # first input DMAs are in flight.
            wsrc = s2pool.tile([128, B], BF16, tag="wsrc")
            nc.gpsimd.memset(wsrc[:], 0.0)
            wps = psum1.tile([128, B], F32, tag="acc7", name="wps")
            for i in range(WARMUP_MM):
                nc.tensor.matmul(
                    wps[:],
                    wsrc[:, 0:128],
                    wsrc[:],
                    start=(i == 0),
                    stop=(i == WARMUP_MM - 1),
                )

            # ---------------- stage 1: partial W.T @ flat.T ----------------
            acc = [
                psum1.tile([128, B], F32, tag=f"acc{i}", name=f"acc_{i}")
                for i in range(4)
            ]
            ccin = dpool.tile(
                [NCORES, 128, 4, BCHUNK], BF16, tag="ccin", name="ccin"
            )
            ccout = dpool.tile(
                [NCORES, 128, 4, BCHUNK], BF16, tag="ccout", name="ccout"
            )

            NK = NKG * KB_PER_G
            for kg in range(NKG):
                xg = xpool.tile([128, KB_PER_G, B], BF16, tag="xg")
                nc.sync.dma_start(out=xg[:], in_=memT[kg])
                wt = wpool.tile([128, KB_PER_G, 2, H], BF16, tag="wt")
                nc.scalar.dma_start(out=wt[:], in_=wpk[kg])
                for kb in range(KB_PER_G):
                    k = kg * KB_PER_G + kb
                    rhs = xg[:, kb, :]
                    for w in range(2):
                        for h in range(2):
                            nc.tensor.matmul(
                                acc[2 * w + h][:],
                                wt[:, kb, w, bass.ts(h, 128)],
                                rhs,
                                start=(k == 0),
                                stop=(k == NK - 1),
                            )
                if kg == 1:
                    # Pre-warm the Tanh/Sigmoid activation tables while the
                    # scalar engine is otherwise idle (first use of each
                    # table pays a ~1.3us load).
                    warm = s2pool.tile([128, 1], F32, tag="warm")
                    nc.gpsimd.memset(warm[:], 0.0)
                    warm2 = s2pool.tile([128, 1], F32, tag="warm2")
                    nc.scalar.activation(warm2[:], warm[:], AF.Tanh)
                    nc.scalar.activation(warm[:], warm2[:], AF.Sigmoid)

            # PSUM -> SBUF (cast bf16, c-major) -> DRAM -> single AllToAll
            sb = ppool.tile([128, NCORES, 4, BCHUNK], BF16, tag="sb", name="sb")
            for t in range(4):
                nc.vector.tensor_copy(
                    sb[:, :, t, :],
                    acc[t][:].rearrange("p (c b) -> p c b", c=NCORES),
                )
            nc.sync.dma_start(
                out=ccin[:].rearrange("c p t b -> p c (t b)"),
                in_=sb[:].rearrange("p c t b -> p c (t b)"),
            )
            nc.gpsimd.collective_compute(
                "AllToAll",
                mybir.AluOpType.bypass,
                replica_groups=RG,
                ins=[ccin[:].opt()],
                outs=[ccout[:].opt()],
            )

            # ------------- local reduction of received slabs -------------
            rr = s2pool.tile([128, NCORES, 4, BCHUNK], BF16, tag="rr", name="rr")
            nc.sync.dma_start(
                out=rr[:].rearrange("p c t b -> p c (t b)"),
                in_=ccout[:].rearrange("c p t b -> p c (t b)"),
            )
            t1 = s2pool.tile([128, 4, 4, BCHUNK], F32, tag="t1", name="t1")
            nc.vector.tensor_add(t1[:], rr[:, 0:4, :, :], rr[:, 4:8, :, :])
            t2 = s2pool.tile([128, 2, 4, BCHUNK], F32, tag="t2", name="t2")
            nc.vector.tensor_add(t2[:], t1[:, 0:2, :, :], t1[:, 2:4, :, :])
            s2 = s2pool.tile([128, 4, BCHUNK], F32, tag="s2in")
            nc.vector.tensor_add(s2[:], t2[:, 0, :, :], t2[:, 1, :, :])

            # ---------------- stage 2: gate + W2 ----------------
            hT = []
            for i in range(2):
                th = s2pool.tile([128, BCHUNK], BF16, tag=f"th{i}", name=f"th{i}")
                nc.scalar.activation(th[:], s2[:, i, :], AF.Tanh, bias=bt[:, i : i + 1])
                sg = s2pool.tile([128, BCHUNK], BF16, tag=f"sg{i}", name=f"sg{i}")
                nc.scalar.activation(
                    sg[:], s2[:, 2 + i, :], AF.Sigmoid, bias=bt[:, 2 + i : 3 + i]
                )
                ht = s2pool.tile([128, BCHUNK], BF16, tag=f"ht{i}", name=f"ht{i}")
                nc.vector.tensor_mul(ht[:], th[:], sg[:])
                hT.append(ht)

            for v in range(2):
                ps = psum1.tile([128, BCHUNK], F32, tag=f"acc{v}", name=f"ps2_{v}")
                for i in range(2):
                    nc.tensor.matmul(
                        ps[:],
                        w2t[:, i, bass.ts(v, 128)],
                        hT[i][:],
                        start=(i == 0),
                        stop=(i == 1),
                    )
                ot = s2pool.tile([128, BCHUNK], F32, tag=f"ot{v}", name=f"ot{v}")
                nc.vector.tensor_scalar_add(ot[:], ps[:], bt[:, 4 + v : 5 + v])
                nc.sync.dma_start(out=outT[bass.ts(v, 128), :], in_=ot[:])

    nc.compile()
    return nc


def _shard(x, memory, W1, b1, Wg, bg, W2, b2):
    """Build the 8 per-core input maps from the full problem inputs."""
    import ml_dtypes

    bf16 = ml_dtypes.bfloat16
    x = np.asarray(x, dtype=np.float32)
    memory = np.asarray(memory, dtype=np.float32)
    W1 = np.asarray(W1, dtype=np.float32)
    Wg = np.asarray(Wg, dtype=np.float32)
    W2 = np.asarray(W2, dtype=np.float32)
    b1 = np.asarray(b1, dtype=np.float32)
    bg = np.asarray(bg, dtype=np.float32)
    b2 = np.asarray(b2, dtype=np.float32)

    # rolled ring buffer, flattened and transposed: [IN, B]
    flatT = np.empty((IN, B), dtype=bf16)
    flatT[:V] = x.T
    flatT[V:] = memory[:, : M - 1, :].reshape(B, IN - V).T
    bpk = np.ascontiguousarray(
        np.stack([b1[:128], b1[128:], bg[:128], bg[128:], b2[:128], b2[128:]], axis=1)
    )
    w2pk = np.ascontiguousarray(
        W2.reshape(2, 128, V).transpose(1, 0, 2).astype(bf16)
    )

    in_maps = []
    for c in range(NCORES):
        sl = slice(KC * c, KC * (c + 1))
        # [NKG, KB, 128, B] -> partition-major [NKG, 128, KB, B]
        mT = np.ascontiguousarray(
            flatT[sl].reshape(NKG, KB_PER_G, 128, B).transpose(0, 2, 1, 3)
        )
        w1s = W1[sl].reshape(NKG, KB_PER_G, 128, H)
        wgs = Wg[sl].reshape(NKG, KB_PER_G, 128, H)
        wpk = np.ascontiguousarray(
            np.stack([w1s, wgs], axis=3).transpose(0, 2, 1, 3, 4).astype(bf16)
        )
        in_maps.append(
            {"memT": mT, "wpk": wpk, "w2pk": w2pk, "bpk": bpk}
        )
    return in_maps


def _get_nc():
    if "nc" not in _CACHE:
        _CACHE["nc"] = _build()
    return _CACHE["nc"]


def kernel(x, memory, W1, b1, Wg, bg, W2, b2, **run_kwargs):
    nc = _get_nc()
    in_maps = _shard(x, memory, W1, b1, Wg, bg, W2, b2)
    res = bass_utils.run_bass_kernel_spmd(
        nc, in_maps, core_ids=list(range(NCORES)), **run_kwargs
    )
    _CACHE["last_results"] = res
    out = np.empty((B, V), dtype=np.float32)
    for c in range(NCORES):
        out[c * BCHUNK : (c + 1) * BCHUNK, :] = res.results[c]["outT"].T
    return out


# revision 8
# speedup vs baseline: 1.2854x; 1.1960x over previous
"""Trainium2 Bass kernel for the gated-MLP-over-ring-buffer problem.

Reference computation (B=512, M=128, V=256, H=256, IN = M*V = 32768):
    mem    = roll(memory, 1, axis=1); mem[:, 0, :] = x        # [B, M, V]
    flat   = mem.reshape(B, IN)                                # [B, 32768]
    h      = tanh(flat @ W1 + b1) * sigmoid(flat @ Wg + bg)    # [B, 256]
    logits = h @ W2 + b2                                       # [B, 256]

Strategy (8 NeuronCores, one trn2 chip):
  - Contraction-shard the two big GEMMs: core c owns k-rows
    [4096c, 4096(c+1)) of W1/Wg and the matching slab of flat.T
    (host-prepared, transposed + bf16 so SBUF tiles load at line rate,
    partition-major so each k-group is one contiguous DMA).
  - Each core computes partial P1.T / Pg.T = W.T @ flat.T  -> [H, B]
    accumulated over its 32 k-chunks in PSUM (bf16 operands, f32 acc).
  - Cross-core reduction of the [2H, B] partials in bf16, scattered
    over B (AllToAll) so core c ends up with batch cols [64c, 64c+64).
    Split in two K-halves so the first AllToAll hides under the second
    half of compute; a dummy AllToAll at kernel start absorbs the
    one-time collective barrier / control-plane warmup.
  - Each core applies bias + tanh/sigmoid gating and the small W2
    GEMM (bf16) for its batch chunk, writing logits.T [V, 64].
  - Host assembles/transposes the 8 chunks back to [B, V].
"""

import numpy as np

import concourse.bacc as bacc
import concourse.bass as bass
import concourse.mybir as mybir
import concourse.tile as tile
from concourse import bass_utils

B, M, V, H = 512, 128, 256, 256
IN = M * V              # 32768
NCORES = 8
KC = IN // NCORES       # 4096 contraction rows per core
NKG = 8                 # DMA k-groups per core
KB_PER_G = KC // (NKG * 128)  # 4 k-chunks of 128 per group
BCHUNK = B // NCORES    # 64 batch columns per core after reduce-scatter
SPLIT_G = 4             # k-groups in the first (early-flushed) half
WARMUP_MM = 12

F32 = mybir.dt.float32
BF16 = mybir.dt.bfloat16
AF = mybir.ActivationFunctionType
RG = [list(range(NCORES))]

_CACHE = {}


def _build():
    nc = bacc.Bacc(
        "TRN2",
        target_bir_lowering=False,
        debug=False,
        enable_asserts=False,
        num_devices=NCORES,
    )

    # Per-core external inputs (host-packed, partition-major, bf16).
    memT = nc.dram_tensor("memT", [NKG, 128, KB_PER_G, B], BF16, kind="ExternalInput")
    wpk = nc.dram_tensor("wpk", [NKG, 128, KB_PER_G, 2, H], BF16, kind="ExternalInput")
    w2pk = nc.dram_tensor("w2pk", [128, 2, V], BF16, kind="ExternalInput")
    # packed biases: cols = [b1_lo, b1_hi, bg_lo, bg_hi, b2_lo, b2_hi]
    bpk = nc.dram_tensor("bpk", [128, 6], F32, kind="ExternalInput")
    outT = nc.dram_tensor("outT", [V, BCHUNK], F32, kind="ExternalOutput")

    with tile.TileContext(nc) as tc:
        with (
            tc.tile_pool(name="xg", bufs=NKG) as xpool,
            tc.tile_pool(name="wt", bufs=NKG) as wpool,
            tc.tile_pool(name="part", bufs=1) as ppool,
            tc.tile_pool(name="s2", bufs=1) as s2pool,
            tc.tile_pool(name="psum1", bufs=1, space="PSUM") as psum1,
            tc.tile_pool(name="dram", bufs=1, space="DRAM") as dpool,
        ):
            # ---- dummy collective, triggered ASAP: the pre-collective
            # rank barrier only completes once EVERY rank has triggered its
            # first collective, so a tiny early AllToAll pulls the barrier
            # (and the ~11us ncfw startup) under the DMA/compute phase.
            dumin = dpool.tile([NCORES, 64], BF16, tag="dumin", name="dumin")
            dumout = dpool.tile([NCORES, 64], BF16, tag="dumout", name="dumout")
            dum = s2pool.tile([NCORES, 64], BF16, tag="dum")
            nc.gpsimd.memset(dum[:], 0.0)
            nc.sync.dma_start(out=dumin[:], in_=dum[:])
            nc.gpsimd.collective_compute(
                "AllToAll",
                mybir.AluOpType.bypass,
                replica_groups=RG,
                ins=[dumin[:].opt()],
                outs=[dumout[:].opt()],
            )

            # Stage-2 constants on the (otherwise idle) gpsimd SWDGE queue.
            bt = s2pool.tile([128, 6], F32, tag="bias")
            nc.gpsimd.dma_start(out=bt[:], in_=bpk[:, :])
            w2t = s2pool.tile([128, 2, V], BF16, tag="w2")
            nc.gpsimd.dma_start(out=w2t[:], in_=w2pk[:, :, :])

            # Pre-warm the PE HAM clock gate with dummy matmuls while the
            # first input DMAs are in flight.
            wsrc = s2pool.tile([128, B], BF16, tag="wsrc")
            nc.gpsimd.memset(wsrc[:], 0.0)
            wps = psum1.tile([128, B], F32, tag="acc7", name="wps")
            for i in range(WARMUP_MM):
                nc.tensor.matmul(
                    wps[:],
                    wsrc[:, 0:128],
                    wsrc[:],
                    start=(i == 0),
                    stop=(i == WARMUP_MM - 1),
                )

            # ---------------- stage 1: partial W.T @ flat.T ----------------
            acc = [
                psum1.tile([128, B], F32, tag=f"acc{i}", name=f"acc_{i}")
                for i in range(4)
            ]
            ccin = dpool.tile(
                [NCORES, 128, 4, BCHUNK], BF16, tag="ccin", name="ccin"
            )
            ccout = dpool.tile(
                [NCORES, 128, 4, BCHUNK], BF16, tag="ccout", name="ccout"
            )

            NK = NKG * KB_PER_G
            for kg in range(NKG):
                xg = xpool.tile([128, KB_PER_G, B], BF16, tag="xg")
                nc.sync.dma_start(out=xg[:], in_=memT[kg])
                wt = wpool.tile([128, KB_PER_G, 2, H], BF16, tag="wt")
                nc.scalar.dma_start(out=wt[:], in_=wpk[kg])
                for kb in range(KB_PER_G):
                    k = kg * KB_PER_G + kb
                    rhs = xg[:, kb, :]
                    for w in range(2):
                        for h in range(2):
                            nc.tensor.matmul(
                                acc[2 * w + h][:],
                                wt[:, kb, w, bass.ts(h, 128)],
                                rhs,
                                start=(k == 0),
                                stop=(k == NK - 1),
                            )
                if kg == 1:
                    # Pre-warm the Tanh/Sigmoid activation tables while the
                    # scalar engine is otherwise idle (first use of each
                    # table pays a ~1.3us load).
                    warm = s2pool.tile([128, 1], F32, tag="warm")
                    nc.gpsimd.memset(warm[:], 0.0)
                    warm2 = s2pool.tile([128, 1], F32, tag="warm2")
                    nc.scalar.activation(warm2[:], warm[:], AF.Tanh)
                    nc.scalar.activation(warm[:], warm2[:], AF.Sigmoid)

            # PSUM -> SBUF (cast bf16, c-major) -> DRAM -> single AllToAll
            sb = ppool.tile([128, NCORES, 4, BCHUNK], BF16, tag="sb", name="sb")
            for t in range(4):
                nc.vector.tensor_copy(
                    sb[:, :, t, :],
                    acc[t][:].rearrange("p (c b) -> p c b", c=NCORES),
                )
            nc.sync.dma_start(
                out=ccin[:].rearrange("c p t b -> p c (t b)"),
                in_=sb[:].rearrange("p c t b -> p c (t b)"),
            )
            nc.gpsimd.collective_compute(
                "AllToAll",
                mybir.AluOpType.bypass,
                replica_groups=RG,
                ins=[ccin[:].opt()],
                outs=[ccout[:].opt()],
            )

            # ------------- local reduction of received slabs -------------
            # Two parallel DMAs (sync + scalar HWDGE queues) halve the drain.
            rr = s2pool.tile([128, NCORES, 4, BCHUNK], BF16, tag="rr", name="rr")
            nc.scalar.dma_start(
                out=rr[:, 0:4].rearrange("p c t b -> p c (t b)"),
                in_=ccout[0:4].rearrange("c p t b -> p c (t b)"),
            )
            nc.sync.dma_start(
                out=rr[:, 4:8].rearrange("p c t b -> p c (t b)"),
                in_=ccout[4:8].rearrange("c p t b -> p c (t b)"),
            )
            t1 = s2pool.tile([128, 4, 4, BCHUNK], BF16, tag="t1", name="t1")
            nc.vector.tensor_add(t1[:], rr[:, 0:4, :, :], rr[:, 4:8, :, :])
            t2 = s2pool.tile([128, 2, 4, BCHUNK], BF16, tag="t2", name="t2")
            nc.vector.tensor_add(t2[:], t1[:, 0:2, :, :], t1[:, 2:4, :, :])
            s2 = s2pool.tile([128, 4, BCHUNK], F32, tag="s2in")
            nc.vector.tensor_add(s2[:], t2[:, 0, :, :], t2[:, 1, :, :])

            # ---------------- stage 2: gate + W2 ----------------
            hT = []
            for i in range(2):
                th = s2pool.tile([128, BCHUNK], BF16, tag=f"th{i}", name=f"th{i}")
                nc.scalar.activation(th[:], s2[:, i, :], AF.Tanh, bias=bt[:, i : i + 1])
                sg = s2pool.tile([128, BCHUNK], BF16, tag=f"sg{i}", name=f"sg{i}")
                nc.scalar.activation(
                    sg[:], s2[:, 2 + i, :], AF.Sigmoid, bias=bt[:, 2 + i : 3 + i]
                )
                ht = s2pool.tile([128, BCHUNK], BF16, tag=f"ht{i}", name=f"ht{i}")
                nc.vector.tensor_mul(ht[:], th[:], sg[:])
                hT.append(ht)

            for v in range(2):
                ps = psum1.tile([128, BCHUNK], F32, tag=f"acc{v}", name=f"ps2_{v}")
                for i in range(2):
                    nc.tensor.matmul(
                        ps[:],
                        w2t[:, i, bass.ts(v, 128)],
                        hT[i][:],
                        start=(i == 0),
                        stop=(i == 1),
                    )
                ot = s2pool.tile([128, BCHUNK], F32, tag=f"ot{v}", name=f"ot{v}")
                nc.vector.tensor_scalar_add(ot[:], ps[:], bt[:, 4 + v : 5 + v])
                nc.sync.dma_start(out=outT[bass.ts(v, 128), :], in_=ot[:])

    nc.compile()
    return nc


def _shard(x, memory, W1, b1, Wg, bg, W2, b2):
    """Build the 8 per-core input maps from the full problem inputs."""
    import ml_dtypes

    bf16 = ml_dtypes.bfloat16
    x = np.asarray(x, dtype=np.float32)
    memory = np.asarray(memory, dtype=np.float32)
    W1 = np.asarray(W1, dtype=np.float32)
    Wg = np.asarray(Wg, dtype=np.float32)
    W2 = np.asarray(W2, dtype=np.float32)
    b1 = np.asarray(b1, dtype=np.float32)
    bg = np.asarray(bg, dtype=np.float32)
    b2 = np.asarray(b2, dtype=np.float32)

    # rolled ring buffer, flattened and transposed: [IN, B]
    flatT = np.empty((IN, B), dtype=bf16)
    flatT[:V] = x.T
    flatT[V:] = memory[:, : M - 1, :].reshape(B, IN - V).T
    bpk = np.ascontiguousarray(
        np.stack([b1[:128], b1[128:], bg[:128], bg[128:], b2[:128], b2[128:]], axis=1)
    )
    w2pk = np.ascontiguousarray(
        W2.reshape(2, 128, V).transpose(1, 0, 2).astype(bf16)
    )

    in_maps = []
    for c in range(NCORES):
        sl = slice(KC * c, KC * (c + 1))
        # [NKG, KB, 128, B] -> partition-major [NKG, 128, KB, B]
        mT = np.ascontiguousarray(
            flatT[sl].reshape(NKG, KB_PER_G, 128, B).transpose(0, 2, 1, 3)
        )
        w1s = W1[sl].reshape(NKG, KB_PER_G, 128, H)
        wgs = Wg[sl].reshape(NKG, KB_PER_G, 128, H)
        wpk = np.ascontiguousarray(
            np.stack([w1s, wgs], axis=3).transpose(0, 2, 1, 3, 4).astype(bf16)
        )
        in_maps.append(
            {"memT": mT, "wpk": wpk, "w2pk": w2pk, "bpk": bpk}
        )
    return in_maps


def _get_nc():
    if "nc" not in _CACHE:
        _CACHE["nc"] = _build()
    return _CACHE["nc"]


def kernel(x, memory, W1, b1, Wg, bg, W2, b2, **run_kwargs):
    nc = _get_nc()
    in_maps = _shard(x, memory, W1, b1, Wg, bg, W2, b2)
    res = bass_utils.run_bass_kernel_spmd(
        nc, in_maps, core_ids=list(range(NCORES)), **run_kwargs
    )
    _CACHE["last_results"] = res
    out = np.empty((B, V), dtype=np.float32)
    for c in range(NCORES):
        out[c * BCHUNK : (c + 1) * BCHUNK, :] = res.results[c]["outT"].T
    return out
